# revision 3
# baseline (speedup 1.0000x reference)
"""AdaptiveFilterAttention on 8 TRN2 NeuronCores.

Sharding: 32 (batch, head) pairs -> 8 cores; core c handles batch c//4,
local head group c%4 (4 heads). Per core: QKV projections for its 256
output dims, per-head attention with exp(-alpha*|i-j|) decay folded in
via rank-1 row scalings of q/k (decay factors exp(+-alpha*t) multiply q
and k rows; diagonal-crossing tiles get a fixup multiply), softmax
without max-subtraction (scores are bounded small), attn@v with an
appended ones-column producing the softmax denominators for free, and a
row-parallel output projection producing a partial (T, D) result summed
on the host across the 4 cores of each batch.
"""
import os
import sys

import numpy as np
import ml_dtypes

sys.path.insert(0, "/opt/trn_rl_repo")

import concourse.bass as bass  # noqa: E402
import concourse.mybir as mybir  # noqa: E402
import concourse.tile as tile  # noqa: E402
from concourse import bacc  # noqa: E402
from concourse.bass_utils import run_bass_kernel_spmd  # noqa: E402

BF16 = mybir.dt.bfloat16
F32 = mybir.dt.float32
P = 128
B, T, D = 2, 2048, 1024
H, HD = 16, 64
HPC = 4            # heads per core
MPC = HD * HPC // P  # 2: partition-tiles of this core's 256 proj dims
NQ = 512           # q free-tile
NT = T // NQ       # 4
KBLK = T // P      # 16 k-blocks of 128
NCORES = 8
DT_CONST = 1.0

LAST_EXEC_NS = None
_GRAPH_CACHE = {}


def _build(kp):
    """Build the per-core Bass graph. kp = number of 128-row contraction
    tiles in the projections (8 without bias row, 9 with)."""
    nc = bacc.Bacc(None, target_bir_lowering=False)

    xT_ext = nc.declare_dram_parameter("xT", [kp * P, T], BF16, isOutput=False)
    wq_ext = nc.declare_dram_parameter("wq", [kp * P, 256], BF16, isOutput=False)
    wk_ext = nc.declare_dram_parameter("wk", [kp * P, 256], BF16, isOutput=False)
    wv_ext = nc.declare_dram_parameter("wv", [kp * P, 256], BF16, isOutput=False)
    wo_ext = nc.declare_dram_parameter("wo", [256, D], BF16, isOutput=False)
    rqlo_ext = nc.declare_dram_parameter("rqlo", [P, T], F32, isOutput=False)
    rqhi_ext = nc.declare_dram_parameter("rqhi", [P, T], F32, isOutput=False)
    rklo_ext = nc.declare_dram_parameter("rklo", [P, T], F32, isOutput=False)
    rkhi_ext = nc.declare_dram_parameter("rkhi", [P, T], F32, isOutput=False)
    corr_ext = nc.declare_dram_parameter("corr", [P, T], F32, isOutput=False)
    out_ext = nc.declare_dram_parameter("out", [D, T], F32, isOutput=True)

    with tile.TileContext(nc) as tc:
        with tc.tile_pool(name="consts", bufs=1) as consts, \
             tc.tile_pool(name="vars", bufs=1) as vars_p, \
             tc.tile_pool(name="dram", bufs=8, space="DRAM") as dram_p:

            xt_sb = consts.tile([P, kp, T], BF16)
            wq_sb = consts.tile([P, kp, 256], BF16)
            wk_sb = consts.tile([P, kp, 256], BF16)
            wv_sb = consts.tile([P, kp, 256], BF16)
            wo_sb = consts.tile([P, 2, D], BF16)
            rqlo = consts.tile([P, T], F32)
            rqhi = consts.tile([P, T], F32)
            rklo = consts.tile([P, T], F32)
            rkhi = consts.tile([P, T], F32)
            corr_sb = consts.tile([P, T], F32)

            for kt in range(kp):
                nc.sync.dma_start(xt_sb[:, kt, :], xT_ext[kt * P:(kt + 1) * P, :])
                nc.sync.dma_start(wq_sb[:, kt, :], wq_ext[kt * P:(kt + 1) * P, :])
                nc.sync.dma_start(wk_sb[:, kt, :], wk_ext[kt * P:(kt + 1) * P, :])
                nc.sync.dma_start(wv_sb[:, kt, :], wv_ext[kt * P:(kt + 1) * P, :])
            for kt2 in range(2):
                nc.sync.dma_start(wo_sb[:, kt2, :], wo_ext[kt2 * P:(kt2 + 1) * P, :])
            nc.sync.dma_start(rqlo[:], rqlo_ext[:])
            nc.sync.dma_start(rqhi[:], rqhi_ext[:])
            nc.sync.dma_start(rklo[:], rklo_ext[:])
            nc.sync.dma_start(rkhi[:], rkhi_ext[:])
            nc.sync.dma_start(corr_sb[:], corr_ext[:])

            # persistent per-core tensors
            q_lo = vars_p.tile([P, MPC, T], BF16)
            q_hi = vars_p.tile([P, MPC, T], BF16)
            k_lo = vars_p.tile([P, MPC, T], BF16)
            k_hi = vars_p.tile([P, MPC, T], BF16)
            v_sb = vars_p.tile([P, KBLK, HPC, HD + 1], BF16)
            o_all = vars_p.tile([P, MPC, T], BF16)

            nc.vector.memset(v_sb[:, :, :, HD:HD + 1], 1.0)

            # ---- Stage A: projections -------------------------------------
            with tc.tile_pool(name="psA", bufs=4, space="PSUM") as psA, \
                 tc.tile_pool(name="psV", bufs=2, space="PSUM") as psV:
                for w_t, lo_r, hi_r, lo_d, hi_d in (
                    (wq_sb, rqlo, rqhi, q_lo, q_hi),
                    (wk_sb, rklo, rkhi, k_lo, k_hi),
                ):
                    for mt in range(MPC):
                        pts = [psA.tile([P, NQ], F32, tag="projps", name=f"pt{_n}")
                               for _n in range(NT)]
                        for kt in range(kp):
                            for nt in range(NT):
                                nc.tensor.matmul(
                                    pts[nt],
                                    w_t[:, kt, mt * P:(mt + 1) * P],
                                    xt_sb[:, kt, nt * NQ:(nt + 1) * NQ],
                                    start=(kt == 0), stop=(kt == kp - 1),
                                )
                        for nt in range(NT):
                            sl = slice(nt * NQ, (nt + 1) * NQ)
                            nc.vector.tensor_tensor(
                                lo_d[:, mt, sl], pts[nt], lo_r[:, sl],
                                mybir.AluOpType.mult)
                            nc.vector.tensor_tensor(
                                hi_d[:, mt, sl], pts[nt], hi_r[:, sl],
                                mybir.AluOpType.mult)
                # v projection: x^T-stationary so v lands [token, dim]
                for mt in range(KBLK):
                    pv = psV.tile([P, 256], F32, tag="vps")
                    for kt in range(kp):
                        nc.tensor.matmul(
                            pv,
                            xt_sb[:, kt, mt * P:(mt + 1) * P],
                            wv_sb[:, kt, :],
                            start=(kt == 0), stop=(kt == kp - 1),
                        )
                    nc.vector.tensor_copy(
                        v_sb[:, mt, :, 0:HD],
                        pv.rearrange("p (h d) -> p h d", h=HPC),
                    )

            # ---- Stage B: attention ---------------------------------------
            with tc.tile_pool(name="spool", bufs=3, space="PSUM") as spool, \
                 tc.tile_pool(name="opool", bufs=2, space="PSUM") as opool, \
                 tc.tile_pool(name="epool", bufs=4) as epool, \
                 tc.tile_pool(name="npool", bufs=4) as npool:
                for pg in range(MPC):
                    for qt in range(NT):
                        qsl = slice(qt * NQ, (qt + 1) * NQ)
                        ops = [opool.tile([P, NQ], F32, tag="ops", name=f"op{_n}")
                               for _n in range(2)]
                        for c in range(T // 256):  # chunks of 2 k-blocks
                            if c < 2 * qt:
                                cls = "lo"
                            elif c <= 2 * qt + 1:
                                cls = "cross"
                            else:
                                cls = "hi"
                            qv, kv = ((q_lo, k_lo) if cls != "hi"
                                      else (q_hi, k_hi))
                            pss = [spool.tile([P, 2 * NQ], F32, tag="spool", name=f"ps{_n}")
                                   for _n in range(2)]
                            for j in range(2):
                                kb = 2 * c + j
                                ksl = slice(kb * P, (kb + 1) * P)
                                for x in range(2):
                                    psl = slice(x * HD, (x + 1) * HD)
                                    nc.tensor.matmul(
                                        pss[x][:, j * NQ:(j + 1) * NQ],
                                        kv[psl, pg, ksl],
                                        qv[psl, pg, qsl],
                                        start=True, stop=True,
                                    )
                            for x in range(2):
                                if cls == "cross":
                                    off = (c - 2 * qt) * 2 * NQ
                                    nc.vector.tensor_tensor(
                                        pss[x][:], pss[x][:],
                                        corr_sb[:, off:off + 2 * NQ],
                                        mybir.AluOpType.mult)
                                e_t = epool.tile([P, 2 * NQ], BF16, tag="e")
                                nc.scalar.activation(
                                    e_t[:], pss[x][:],
                                    mybir.ActivationFunctionType.Exp)
                                for j in range(2):
                                    kb = 2 * c + j
                                    nc.tensor.matmul(
                                        ops[x][0:HD + 1, :],
                                        v_sb[:, kb, 2 * pg + x, :],
                                        e_t[:, j * NQ:(j + 1) * NQ],
                                        start=(c == 0 and j == 0),
                                        stop=(c == T // 256 - 1 and j == 1),
                                    )
                        # normalization: sums live in row HD of ops[x]
                        for x in range(2):
                            sums_sb = npool.tile([P, NQ], F32, tag="sums")
                            nc.any.tensor_copy(sums_sb[HD:HD + 1, :],
                                               ops[x][HD:HD + 1, :])
                            dsum = dram_p.tile([1, NQ], F32, tag="dsum")
                            nc.sync.dma_start(dsum[:], sums_sb[HD:HD + 1, :])
                            srep = npool.tile([HD, NQ], F32, tag="srep")
                            nc.sync.dma_start(
                                srep[:], dsum[:].to_broadcast((HD, NQ)))
                            rrep = npool.tile([HD, NQ], F32, tag="rrep")
                            nc.vector.reciprocal_approx_fast(rrep[:], srep[:])
                            if x == 0:
                                nc.vector.tensor_tensor(
                                    o_all[0:HD, pg, qsl], ops[x][0:HD, :],
                                    rrep[:], mybir.AluOpType.mult)
                            else:
                                ob = npool.tile([HD, NQ], BF16, tag="ob")
                                nc.vector.tensor_tensor(
                                    ob[:], ops[x][0:HD, :], rrep[:],
                                    mybir.AluOpType.mult)
                                nc.sync.dma_start(o_all[HD:P, pg, qsl], ob[:])

            # ---- Stage C: output projection -------------------------------
            with tc.tile_pool(name="cpool", bufs=4, space="PSUM") as cpool, \
                 tc.tile_pool(name="fpool", bufs=4) as fpool:
                for mt in range(D // P):
                    for nt in range(NT):
                        pc = cpool.tile([P, NQ], F32, tag="cps")
                        for kt2 in range(2):
                            nc.tensor.matmul(
                                pc,
                                wo_sb[:, kt2, mt * P:(mt + 1) * P],
                                o_all[:, kt2, nt * NQ:(nt + 1) * NQ],
                                start=(kt2 == 0), stop=(kt2 == 1),
                            )
                        fo = fpool.tile([P, NQ], F32, tag="fo")
                        nc.any.tensor_copy(fo[:], pc[:])
                        nc.sync.dma_start(
                            out_ext[mt * P:(mt + 1) * P,
                                    nt * NQ:(nt + 1) * NQ],
                            fo[:])

    nc.finalize()
    return nc


def _get_graph(kp):
    if kp not in _GRAPH_CACHE:
        _GRAPH_CACHE[kp] = _build(kp)
    return _GRAPH_CACHE[kp]


def _install_trace_hooks():
    import types
    import antenv
    if "antenv.axon_hooks" not in sys.modules:
        hooks = types.ModuleType("antenv.axon_hooks")
        hooks._hook = None
        hooks.set_axon_ntff_profile_hook = lambda h: setattr(hooks, "_hook", h)
        hooks.get_axon_ntff_profile_hook = lambda: hooks._hook
        sys.modules["antenv.axon_hooks"] = hooks
        antenv.axon_hooks = hooks
    if sys.modules["antenv.axon_hooks"]._hook is None:
        if "/root/.axon_site" not in sys.path:
            sys.path.insert(0, "/root/.axon_site")
        from trn_agent_boot.trn_boot import _ntff_profile_via_ctypes
        sys.modules["antenv.axon_hooks"].set_axon_ntff_profile_hook(
            _ntff_profile_via_ctypes("/opt/axon/libaxon_pjrt.so"))


def kernel(x, Wq, bq, Wk, bk, Wv, bv, Wo, bo, alpha):
    global LAST_EXEC_NS
    x = np.asarray(x, dtype=np.float32)
    Wq = np.asarray(Wq, dtype=np.float32)
    Wk = np.asarray(Wk, dtype=np.float32)
    Wv = np.asarray(Wv, dtype=np.float32)
    Wo = np.asarray(Wo, dtype=np.float32)
    bq = np.asarray(bq, dtype=np.float32)
    bk = np.asarray(bk, dtype=np.float32)
    bv = np.asarray(bv, dtype=np.float32)
    bo = np.asarray(bo, dtype=np.float32)
    alpha = float(np.asarray(alpha))
    a_eff = alpha * DT_CONST
    scale = HD ** -0.5

    has_bias = bool(np.any(bq) or np.any(bk) or np.any(bv))
    kp = 9 if has_bias else 8
    nc = _get_graph(kp)

    t_idx = np.arange(T, dtype=np.float64)
    e_neg = np.exp(-a_eff * t_idx)
    e_pos = np.exp(+a_eff * t_idx)
    rqlo = np.tile((scale * e_neg).astype(np.float32), (P, 1))
    rqhi = np.tile((scale * e_pos).astype(np.float32), (P, 1))
    rklo = np.tile(e_pos.astype(np.float32), (P, 1))
    rkhi = np.tile(e_neg.astype(np.float32), (P, 1))

    # corr[kk, o*512+qq] = 1 if d>=0 else exp(2*a_eff*d), d = qq-kk-128*o
    kk = np.arange(P)[:, None]
    qq = np.arange(NQ)[None, :]
    corr = np.empty((P, T), dtype=np.float32)
    for o in range(4):
        d = qq - kk - P * o
        corr[:, o * NQ:(o + 1) * NQ] = np.where(
            d >= 0, 1.0, np.exp(2.0 * a_eff * d))

    def wslice(W, b, g):
        ws = W[256 * g:256 * g + 256, :].T.astype(np.float64)
        if has_bias:
            ws = np.vstack([ws, b[256 * g:256 * g + 256][None, :],
                            np.zeros((kp * P - D - 1, 256))])
        return np.ascontiguousarray(ws).astype(ml_dtypes.bfloat16)

    in_maps = []
    for core in range(NCORES):
        b_idx, g = core // 4, core % 4
        xT = x[b_idx].T.astype(np.float64)
        if has_bias:
            xT = np.vstack([xT, np.ones((1, T)), np.zeros((kp * P - D - 1, T))])
        in_maps.append({
            "xT": np.ascontiguousarray(xT).astype(ml_dtypes.bfloat16),
            "wq": wslice(Wq, bq, g),
            "wk": wslice(Wk, bk, g),
            "wv": wslice(Wv, bv, g),
            "wo": np.ascontiguousarray(
                Wo[:, 256 * g:256 * g + 256].T).astype(ml_dtypes.bfloat16),
            "rqlo": rqlo, "rqhi": rqhi, "rklo": rklo, "rkhi": rkhi,
            "corr": corr,
        })

    trace = bool(os.environ.get("BASS_KERNEL_TRACE"))
    if trace:
        _install_trace_hooks()
    res = run_bass_kernel_spmd(nc, in_maps, core_ids=list(range(NCORES)),
                               trace=trace)
    LAST_EXEC_NS = res.exec_time_ns

    out = np.empty((B, T, D), dtype=np.float32)
    for b_idx in range(B):
        acc = np.zeros((D, T), dtype=np.float32)
        for g in range(4):
            acc += res.results[b_idx * 4 + g]["out"]
        out[b_idx] = acc.T + bo[None, :]
    return out


# revision 4
# speedup vs baseline: 1.0260x; 1.0260x over previous
"""AdaptiveFilterAttention on 8 TRN2 NeuronCores.

Sharding: 32 (batch, head) pairs -> 8 cores; core c handles batch c//4,
local head group c%4 (4 heads). Per core: QKV projections for its 256
output dims, per-head attention with exp(-alpha*|i-j|) decay folded in
via rank-1 row scalings of q/k (decay factors exp(+-alpha*t) multiply q
and k rows; diagonal-crossing tiles get a fixup multiply), softmax
without max-subtraction (scores are bounded small), attn@v with an
appended ones-column producing the softmax denominators for free, and a
row-parallel output projection producing a partial (T, D) result summed
on the host across the 4 cores of each batch.
"""
import os
import sys

import numpy as np
import ml_dtypes

sys.path.insert(0, "/opt/trn_rl_repo")

import concourse.bass as bass  # noqa: E402
import concourse.mybir as mybir  # noqa: E402
import concourse.tile as tile  # noqa: E402
from concourse import bacc  # noqa: E402
from concourse.bass_utils import run_bass_kernel_spmd  # noqa: E402

BF16 = mybir.dt.bfloat16
F32 = mybir.dt.float32
P = 128
B, T, D = 2, 2048, 1024
H, HD = 16, 64
HPC = 4            # heads per core
MPC = HD * HPC // P  # 2: partition-tiles of this core's 256 proj dims
NQ = 512           # q free-tile
NT = T // NQ       # 4
KBLK = T // P      # 16 k-blocks of 128
NCORES = 8
DT_CONST = 1.0

LAST_EXEC_NS = None
LAST_RESULT = None
_GRAPH_CACHE = {}


def _build(kp):
    """Build the per-core Bass graph. kp = number of 128-row contraction
    tiles in the projections (8 without bias row, 9 with)."""
    nc = bacc.Bacc(None, target_bir_lowering=False)

    xT_ext = nc.declare_dram_parameter("xT", [kp * P, T], BF16, isOutput=False)
    wq_ext = nc.declare_dram_parameter("wq", [kp * P, 256], BF16, isOutput=False)
    wk_ext = nc.declare_dram_parameter("wk", [kp * P, 256], BF16, isOutput=False)
    wv_ext = nc.declare_dram_parameter("wv", [kp * P, 256], BF16, isOutput=False)
    wo_ext = nc.declare_dram_parameter("wo", [256, D], BF16, isOutput=False)
    rqlo_ext = nc.declare_dram_parameter("rqlo", [P, T], F32, isOutput=False)
    rqhi_ext = nc.declare_dram_parameter("rqhi", [P, T], F32, isOutput=False)
    rklo_ext = nc.declare_dram_parameter("rklo", [P, T], F32, isOutput=False)
    rkhi_ext = nc.declare_dram_parameter("rkhi", [P, T], F32, isOutput=False)
    corr_ext = nc.declare_dram_parameter("corr", [P, T], F32, isOutput=False)
    out_ext = nc.declare_dram_parameter("out", [D, T], F32, isOutput=True)

    with tile.TileContext(nc) as tc:
        with tc.tile_pool(name="consts", bufs=1) as consts, \
             tc.tile_pool(name="vars", bufs=1) as vars_p, \
             tc.tile_pool(name="dram", bufs=8, space="DRAM") as dram_p:

            xt_sb = consts.tile([P, kp, T], BF16)
            wq_sb = consts.tile([P, kp, 256], BF16)
            wk_sb = consts.tile([P, kp, 256], BF16)
            wv_sb = consts.tile([P, kp, 256], BF16)
            wo_sb = consts.tile([P, 2, D], BF16)
            rqlo = consts.tile([P, T], F32)
            rqhi = consts.tile([P, T], F32)
            rklo = consts.tile([P, T], F32)
            rkhi = consts.tile([P, T], F32)
            corr_sb = consts.tile([P, T], F32)

            for kt in range(kp):
                nc.sync.dma_start(xt_sb[:, kt, :], xT_ext[kt * P:(kt + 1) * P, :])
                nc.sync.dma_start(wq_sb[:, kt, :], wq_ext[kt * P:(kt + 1) * P, :])
                nc.sync.dma_start(wk_sb[:, kt, :], wk_ext[kt * P:(kt + 1) * P, :])
                nc.sync.dma_start(wv_sb[:, kt, :], wv_ext[kt * P:(kt + 1) * P, :])
            for kt2 in range(2):
                nc.sync.dma_start(wo_sb[:, kt2, :], wo_ext[kt2 * P:(kt2 + 1) * P, :])
            nc.sync.dma_start(rqlo[:], rqlo_ext[:])
            nc.sync.dma_start(rqhi[:], rqhi_ext[:])
            nc.sync.dma_start(rklo[:], rklo_ext[:])
            nc.sync.dma_start(rkhi[:], rkhi_ext[:])
            nc.sync.dma_start(corr_sb[:], corr_ext[:])

            # persistent per-core tensors
            q_lo = vars_p.tile([P, MPC, T], BF16)
            q_hi = vars_p.tile([P, MPC, T], BF16)
            k_lo = vars_p.tile([P, MPC, T], BF16)
            k_hi = vars_p.tile([P, MPC, T], BF16)
            v_sb = vars_p.tile([P, KBLK, HPC, HD + 1], BF16)
            o_all = vars_p.tile([P, MPC, T], BF16)

            nc.vector.memset(v_sb[:, :, :, HD:HD + 1], 1.0)

            # ---- Stage A: projections -------------------------------------
            with tc.tile_pool(name="psA", bufs=4, space="PSUM") as psA, \
                 tc.tile_pool(name="psV", bufs=2, space="PSUM") as psV:
                for w_t, lo_r, hi_r, lo_d, hi_d in (
                    (wq_sb, rqlo, rqhi, q_lo, q_hi),
                    (wk_sb, rklo, rkhi, k_lo, k_hi),
                ):
                    for mt in range(MPC):
                        pts = [psA.tile([P, NQ], F32, tag="projps", name=f"pt{_n}")
                               for _n in range(NT)]
                        for kt in range(kp):
                            for nt in range(NT):
                                nc.tensor.matmul(
                                    pts[nt],
                                    w_t[:, kt, mt * P:(mt + 1) * P],
                                    xt_sb[:, kt, nt * NQ:(nt + 1) * NQ],
                                    start=(kt == 0), stop=(kt == kp - 1),
                                )
                        for nt in range(NT):
                            sl = slice(nt * NQ, (nt + 1) * NQ)
                            nc.vector.tensor_tensor(
                                lo_d[:, mt, sl], pts[nt], lo_r[:, sl],
                                mybir.AluOpType.mult)
                            nc.vector.tensor_tensor(
                                hi_d[:, mt, sl], pts[nt], hi_r[:, sl],
                                mybir.AluOpType.mult)
                # v projection: x^T-stationary so v lands [token, dim]
                for mt in range(KBLK):
                    pv = psV.tile([P, 256], F32, tag="vps")
                    for kt in range(kp):
                        nc.tensor.matmul(
                            pv,
                            xt_sb[:, kt, mt * P:(mt + 1) * P],
                            wv_sb[:, kt, :],
                            start=(kt == 0), stop=(kt == kp - 1),
                        )
                    nc.vector.tensor_copy(
                        v_sb[:, mt, :, 0:HD],
                        pv.rearrange("p (h d) -> p h d", h=HPC),
                    )

            # ---- Stage B: attention ---------------------------------------
            with tc.tile_pool(name="spool", bufs=3, space="PSUM") as spool, \
                 tc.tile_pool(name="opool", bufs=2, space="PSUM") as opool, \
                 tc.tile_pool(name="epool", bufs=4) as epool, \
                 tc.tile_pool(name="npool", bufs=4) as npool:
                for pg in range(MPC):
                    for qt in range(NT):
                        qsl = slice(qt * NQ, (qt + 1) * NQ)
                        ops = [opool.tile([P, NQ], F32, tag="ops", name=f"op{_n}")
                               for _n in range(2)]
                        for c in range(T // 256):  # chunks of 2 k-blocks
                            if c < 2 * qt:
                                cls = "lo"
                            elif c <= 2 * qt + 1:
                                cls = "cross"
                            else:
                                cls = "hi"
                            qv, kv = ((q_lo, k_lo) if cls != "hi"
                                      else (q_hi, k_hi))
                            pss = [spool.tile([P, 2 * NQ], F32, tag="spool", name=f"ps{_n}")
                                   for _n in range(2)]
                            for j in range(2):
                                kb = 2 * c + j
                                ksl = slice(kb * P, (kb + 1) * P)
                                for x in range(2):
                                    psl = slice(x * HD, (x + 1) * HD)
                                    nc.tensor.matmul(
                                        pss[x][:, j * NQ:(j + 1) * NQ],
                                        kv[psl, pg, ksl],
                                        qv[psl, pg, qsl],
                                        start=True, stop=True,
                                    )
                            for x in range(2):
                                if cls == "cross":
                                    off = (c - 2 * qt) * 2 * NQ
                                    nc.vector.tensor_tensor(
                                        pss[x][:], pss[x][:],
                                        corr_sb[:, off:off + 2 * NQ],
                                        mybir.AluOpType.mult)
                                e_t = epool.tile([P, 2 * NQ], BF16, tag="e")
                                nc.scalar.activation(
                                    e_t[:], pss[x][:],
                                    mybir.ActivationFunctionType.Exp)
                                for j in range(2):
                                    kb = 2 * c + j
                                    nc.tensor.matmul(
                                        ops[x][0:HD + 1, :],
                                        v_sb[:, kb, 2 * pg + x, :],
                                        e_t[:, j * NQ:(j + 1) * NQ],
                                        start=(c == 0 and j == 0),
                                        stop=(c == T // 256 - 1 and j == 1),
                                    )
                        # normalization: sums live in row HD of ops[x]
                        for x in range(2):
                            sums_sb = npool.tile([P, NQ], F32, tag="sums")
                            nc.any.tensor_copy(sums_sb[HD:HD + 1, :],
                                               ops[x][HD:HD + 1, :])
                            dsum = dram_p.tile([1, NQ], F32, tag="dsum")
                            nc.sync.dma_start(dsum[:], sums_sb[HD:HD + 1, :])
                            srep = npool.tile([HD, NQ], F32, tag="srep")
                            nc.sync.dma_start(
                                srep[:], dsum[:].to_broadcast((HD, NQ)))
                            rrep = npool.tile([HD, NQ], F32, tag="rrep")
                            nc.vector.reciprocal_approx_fast(rrep[:], srep[:])
                            if x == 0:
                                nc.vector.tensor_tensor(
                                    o_all[0:HD, pg, qsl], ops[x][0:HD, :],
                                    rrep[:], mybir.AluOpType.mult)
                            else:
                                ob = npool.tile([HD, NQ], BF16, tag="ob")
                                nc.vector.tensor_tensor(
                                    ob[:], ops[x][0:HD, :], rrep[:],
                                    mybir.AluOpType.mult)
                                nc.sync.dma_start(o_all[HD:P, pg, qsl], ob[:])

            # ---- Stage C: output projection -------------------------------
            with tc.tile_pool(name="cpool", bufs=4, space="PSUM") as cpool, \
                 tc.tile_pool(name="fpool", bufs=4) as fpool:
                for mt in range(D // P):
                    for nt in range(NT):
                        pc = cpool.tile([P, NQ], F32, tag="cps")
                        for kt2 in range(2):
                            nc.tensor.matmul(
                                pc,
                                wo_sb[:, kt2, mt * P:(mt + 1) * P],
                                o_all[:, kt2, nt * NQ:(nt + 1) * NQ],
                                start=(kt2 == 0), stop=(kt2 == 1),
                            )
                        fo = fpool.tile([P, NQ], F32, tag="fo")
                        nc.any.tensor_copy(fo[:], pc[:])
                        nc.sync.dma_start(
                            out_ext[mt * P:(mt + 1) * P,
                                    nt * NQ:(nt + 1) * NQ],
                            fo[:])

    nc.finalize()
    return nc


def _get_graph(kp):
    if kp not in _GRAPH_CACHE:
        _GRAPH_CACHE[kp] = _build(kp)
    return _GRAPH_CACHE[kp]


def _install_trace_hooks():
    import types
    import antenv
    if "antenv.axon_hooks" not in sys.modules:
        hooks = types.ModuleType("antenv.axon_hooks")
        hooks._hook = None
        hooks.set_axon_ntff_profile_hook = lambda h: setattr(hooks, "_hook", h)
        hooks.get_axon_ntff_profile_hook = lambda: hooks._hook
        sys.modules["antenv.axon_hooks"] = hooks
        antenv.axon_hooks = hooks
    if sys.modules["antenv.axon_hooks"]._hook is None:
        if "/root/.axon_site" not in sys.path:
            sys.path.insert(0, "/root/.axon_site")
        from trn_agent_boot.trn_boot import _ntff_profile_via_ctypes
        sys.modules["antenv.axon_hooks"].set_axon_ntff_profile_hook(
            _ntff_profile_via_ctypes("/opt/axon/libaxon_pjrt.so"))


def kernel(x, Wq, bq, Wk, bk, Wv, bv, Wo, bo, alpha):
    global LAST_EXEC_NS, LAST_RESULT
    x = np.asarray(x, dtype=np.float32)
    Wq = np.asarray(Wq, dtype=np.float32)
    Wk = np.asarray(Wk, dtype=np.float32)
    Wv = np.asarray(Wv, dtype=np.float32)
    Wo = np.asarray(Wo, dtype=np.float32)
    bq = np.asarray(bq, dtype=np.float32)
    bk = np.asarray(bk, dtype=np.float32)
    bv = np.asarray(bv, dtype=np.float32)
    bo = np.asarray(bo, dtype=np.float32)
    alpha = float(np.asarray(alpha))
    a_eff = alpha * DT_CONST
    scale = HD ** -0.5

    has_bias = bool(np.any(bq) or np.any(bk) or np.any(bv))
    kp = 9 if has_bias else 8
    nc = _get_graph(kp)

    t_idx = np.arange(T, dtype=np.float64)
    e_neg = np.exp(-a_eff * t_idx)
    e_pos = np.exp(+a_eff * t_idx)
    rqlo = np.tile((scale * e_neg).astype(np.float32), (P, 1))
    rqhi = np.tile((scale * e_pos).astype(np.float32), (P, 1))
    rklo = np.tile(e_pos.astype(np.float32), (P, 1))
    rkhi = np.tile(e_neg.astype(np.float32), (P, 1))

    # corr[kk, o*512+qq] = 1 if d>=0 else exp(2*a_eff*d), d = qq-kk-128*o
    kk = np.arange(P)[:, None]
    qq = np.arange(NQ)[None, :]
    corr = np.empty((P, T), dtype=np.float32)
    for o in range(4):
        d = qq - kk - P * o
        corr[:, o * NQ:(o + 1) * NQ] = np.where(
            d >= 0, 1.0, np.exp(2.0 * a_eff * d))

    def wslice(W, b, g):
        ws = W[256 * g:256 * g + 256, :].T.astype(np.float64)
        if has_bias:
            ws = np.vstack([ws, b[256 * g:256 * g + 256][None, :],
                            np.zeros((kp * P - D - 1, 256))])
        return np.ascontiguousarray(ws).astype(ml_dtypes.bfloat16)

    in_maps = []
    for core in range(NCORES):
        b_idx, g = core // 4, core % 4
        xT = x[b_idx].T.astype(np.float64)
        if has_bias:
            xT = np.vstack([xT, np.ones((1, T)), np.zeros((kp * P - D - 1, T))])
        in_maps.append({
            "xT": np.ascontiguousarray(xT).astype(ml_dtypes.bfloat16),
            "wq": wslice(Wq, bq, g),
            "wk": wslice(Wk, bk, g),
            "wv": wslice(Wv, bv, g),
            "wo": np.ascontiguousarray(
                Wo[:, 256 * g:256 * g + 256].T).astype(ml_dtypes.bfloat16),
            "rqlo": rqlo, "rqhi": rqhi, "rklo": rklo, "rkhi": rkhi,
            "corr": corr,
        })

    trace = bool(os.environ.get("BASS_KERNEL_TRACE"))
    if trace:
        _install_trace_hooks()
    res = run_bass_kernel_spmd(nc, in_maps, core_ids=list(range(NCORES)),
                               trace=trace)
    LAST_EXEC_NS = res.exec_time_ns
    LAST_RESULT = res

    out = np.empty((B, T, D), dtype=np.float32)
    for b_idx in range(B):
        acc = np.zeros((D, T), dtype=np.float32)
        for g in range(4):
            acc += res.results[b_idx * 4 + g]["out"]
        out[b_idx] = acc.T + bo[None, :]
    return out
